# revision 7
# baseline (speedup 1.0000x reference)
"""ARMIN/TARDIS addressed-memory cell on 8 trn2 NeuronCores.

Data-parallel over batch: each core processes 32 of the 256 batch rows.
Weights are replicated. Score path (content addressing) runs in fp32/f32r
(argmax margins are as small as 4e-4, bf16 would flip reads); the two big
cell matmuls run in bf16 (weights+activations), h_entry is gathered in fp32
via indirect DMA so the r-output stays exact.
"""

import numpy as np
import ml_dtypes
from contextlib import ExitStack

import concourse.bass as bass
import concourse.bacc as bacc
import concourse.tile as tile
from concourse import mybir
from concourse.bass_utils import run_bass_kernel_spmd
from concourse.masks import make_identity

F32 = mybir.dt.float32
F32R = mybir.dt.float32r
BF16 = mybir.dt.bfloat16
U32 = mybir.dt.uint32
AF = mybir.ActivationFunctionType
ALU = mybir.AluOpType
AX = mybir.AxisListType

N_CORES = 8
B, X, H, M, KD = 256, 512, 1024, 128, 64
BC = B // N_CORES        # 32 batch rows per core
F = H // 4               # 256
BM = BC * M              # 4096
EPS = 1e-5
F_BIAS = 1.0
CHUNK = 256              # bm columns per score-path tile
NCHUNK = BM // CHUNK     # 16
KCAT = (X + 2 * H) // 128  # 20 contraction tiles for the cell matmuls


def _bcast_rows(handle_ap, lo, hi, rows=BC):
    """AP that reads dram vector[lo:hi] broadcast across `rows` partitions."""
    src = handle_ap[lo:hi]
    return bass.AP(tensor=src.tensor, offset=src.offset,
                   ap=[[0, rows]] + [list(d) for d in src.ap])


def _ln_rows(nc, bnpool, x_ap, d, g_tile, b_tile, out_ap, eps_t):
    """out = (x - mean)/sqrt(var + EPS) * g + b, row-wise over free dim d."""
    nsub = (d + 511) // 512
    sub = d // nsub
    assert sub * nsub == d and sub <= 512
    stats = bnpool.tile([BC, nsub, 6], F32, tag="bn_stats")
    for i in range(nsub):
        nc.vector.bn_stats(out=stats[:, i, :], in_=x_ap[:, i * sub:(i + 1) * sub])
    mv = bnpool.tile([BC, 2], F32, tag="bn_mv")
    nc.vector.bn_aggr(out=mv[:], in_=stats[:])
    rstd = bnpool.tile([BC, 1], F32, tag="bn_rstd")
    nc.scalar.activation(out=rstd[:], in_=mv[:, 1:2], func=AF.Sqrt,
                         bias=eps_t[:], scale=1.0)
    nc.vector.reciprocal(out=rstd[:], in_=rstd[:])
    nc.vector.tensor_scalar(out=out_ap, in0=x_ap, scalar1=mv[:, 0:1],
                            scalar2=rstd[:], op0=ALU.subtract, op1=ALU.mult)
    nc.vector.tensor_mul(out=out_ap, in0=out_ap, in1=g_tile)
    nc.vector.tensor_add(out=out_ap, in0=out_ap, in1=b_tile)


def build_nc():
    nc = bacc.Bacc("TRN2", target_bir_lowering=False, debug=False)
    P = {}

    def dp(name, shape, dtype=F32, out=False):
        P[name] = nc.declare_dram_parameter(name, list(shape), dtype, isOutput=out)
        return P[name]

    dp("hmemT_hi", [H, BM], BF16)   # [h, b*128+m] bf16 high part
    dp("hmemT_lo", [H, BM], BF16)   # residual (hmemT - hi) in bf16
    dp("hmem_flat", [BM, H])        # natural rows for the gather
    dp("xT", [X, BC]); dp("cT", [H, BC])
    dp("xT_bf", [X, BC], BF16); dp("cT_bf", [H, BC], BF16)
    dp("c_nat", [BC, H])
    dp("u_t", [BC, M]); dp("prev", [BC, M]); dp("gumbel_u", [BC, M])
    dp("keysT_pad", [128, M])
    dp("fc_w", [X + 2 * H + KD + M, F])
    dp("fc_b", [F]); dp("vec_a", [F])
    dp("fchm_hi", [H, F], BF16)     # fc_w rows 1600:2624 hi/lo split
    dp("fchm_lo", [H, F], BF16)
    dp("row_base", [BC, 1], U32)
    dp("w1bf", [X + 2 * H, 2 * H], BF16)
    dp("wfbf", [X + 2 * H, 5 * H], BF16)
    dp("bias1v", [2 * H]); dp("biasv", [5 * H])
    dp("ln1g", [5 * H]); dp("ln1b", [5 * H])
    dp("ln2g", [H]); dp("ln2b", [H])
    dp("ln3g", [2 * H]); dp("ln3b", [2 * H])
    dp("ln4g", [M]); dp("ln4b", [M])
    out_d = dp("out", [BC, 2 * H], out=True)
    score_d = nc.dram_tensor("score_bounce", [BM], F32)

    with ExitStack() as ctx:
        tc = ctx.enter_context(tile.TileContext(nc))
        consts = ctx.enter_context(tc.tile_pool(name="consts", bufs=1))
        hpool = ctx.enter_context(tc.tile_pool(name="hpool", bufs=2))
        wpool = ctx.enter_context(tc.tile_pool(name="wpool", bufs=3))
        hfpool = ctx.enter_context(tc.tile_pool(name="hfpool", bufs=3))
        bnpool = ctx.enter_context(tc.tile_pool(name="bnpool", bufs=4))
        zpool = ctx.enter_context(tc.tile_pool(name="zpool", bufs=2))
        bcpool = ctx.enter_context(tc.tile_pool(name="bcpool", bufs=3))
        pre_ps = ctx.enter_context(tc.tile_pool(name="pre_ps", bufs=2, space="PSUM"))
        sc_ps = ctx.enter_context(tc.tile_pool(name="sc_ps", bufs=1, space="PSUM"))
        tp_ps = ctx.enter_context(tc.tile_pool(name="tp_ps", bufs=2, space="PSUM"))
        cell_ps = ctx.enter_context(tc.tile_pool(name="cell_ps", bufs=1, space="PSUM"))

        # ---------- resident constants ----------
        ident = consts.tile([128, 128], F32, tag="ident")
        make_identity(nc, ident[:])
        eps_t = consts.tile([BC, 1], F32, tag="eps")
        nc.vector.memset(eps_t[:], EPS)
        e20_t = consts.tile([BC, 1], F32, tag="e20")
        nc.vector.memset(e20_t[:], 1e-20)

        fcxc = consts.tile([128, 12, F], F32, tag="fcxc")
        nc.sync.dma_start(out=fcxc[:], in_=P["fc_w"].ap()[0:1536, :]
                          .rearrange("(j p) n -> p j n", p=128))
        fckp = consts.tile([128, F], F32, tag="fckp")
        nc.sync.dma_start(out=fckp[:], in_=P["fc_w"].ap()[1536:1664, :])
        fchm_hi = consts.tile([128, 8, F], BF16, tag="fchm_hi")
        nc.sync.dma_start(out=fchm_hi[:], in_=P["fchm_hi"].ap()[:, :]
                          .rearrange("(j p) n -> p j n", p=128))
        fchm_lo = consts.tile([128, 8, F], BF16, tag="fchm_lo")
        nc.sync.dma_start(out=fchm_lo[:], in_=P["fchm_lo"].ap()[:, :]
                          .rearrange("(j p) n -> p j n", p=128))
        fcu = consts.tile([128, F], F32, tag="fcu")
        nc.sync.dma_start(out=fcu[:], in_=P["fc_w"].ap()[2624:2752, :])
        fcb = consts.tile([128, 2], F32, tag="fcb")
        nc.sync.dma_start(out=fcb[:], in_=P["fc_b"].ap().rearrange("(f p) -> p f", p=128))
        veca = consts.tile([128, 2], F32, tag="veca")
        nc.sync.dma_start(out=veca[:], in_=P["vec_a"].ap().rearrange("(f p) -> p f", p=128))
        keysT = consts.tile([128, M], F32, tag="keysT")
        nc.sync.dma_start(out=keysT[:], in_=P["keysT_pad"].ap()[:])

        xT_f = consts.tile([128, 4, BC], F32, tag="xT_f")
        nc.sync.dma_start(out=xT_f[:], in_=P["xT"].ap()[:, :]
                          .rearrange("(j p) n -> p j n", p=128))
        cT_f = consts.tile([128, 8, BC], F32, tag="cT_f")
        nc.sync.dma_start(out=cT_f[:], in_=P["cT"].ap()[:, :]
                          .rearrange("(j p) n -> p j n", p=128))
        # bf16 stationary tiles for the two cell matmuls (k-tiles 0..19)
        ck_bf = consts.tile([128, KCAT, BC], BF16, tag="ck_bf")    # ungated
        ckg_bf = consts.tile([128, KCAT, BC], BF16, tag="ckg_bf")  # gated
        nc.sync.dma_start(out=ck_bf[:, 0:4, :], in_=P["xT_bf"].ap()[:, :]
                          .rearrange("(j p) n -> p j n", p=128))
        nc.sync.dma_start(out=ckg_bf[:, 0:4, :], in_=P["xT_bf"].ap()[:, :]
                          .rearrange("(j p) n -> p j n", p=128))
        nc.sync.dma_start(out=ck_bf[:, 4:12, :], in_=P["cT_bf"].ap()[:, :]
                          .rearrange("(j p) n -> p j n", p=128))

        c_nat = consts.tile([BC, H], F32, tag="c_nat")
        nc.sync.dma_start(out=c_nat[:], in_=P["c_nat"].ap()[:])
        u_sb = consts.tile([BC, M], F32, tag="u_sb")
        nc.sync.dma_start(out=u_sb[:], in_=P["u_t"].ap()[:])
        prev_sb = consts.tile([BC, M], F32, tag="prev_sb")
        nc.sync.dma_start(out=prev_sb[:], in_=P["prev"].ap()[:])
        gum_sb = consts.tile([BC, M], F32, tag="gum_sb")
        nc.sync.dma_start(out=gum_sb[:], in_=P["gumbel_u"].ap()[:])
        rowb = consts.tile([BC, 1], U32, tag="rowb")
        nc.sync.dma_start(out=rowb[:], in_=P["row_base"].ap()[:])

        ln4g_t = consts.tile([BC, M], F32, tag="ln4g_t")
        nc.gpsimd.dma_start(out=ln4g_t[:], in_=_bcast_rows(P["ln4g"].ap(), 0, M))
        ln4b_t = consts.tile([BC, M], F32, tag="ln4b_t")
        nc.gpsimd.dma_start(out=ln4b_t[:], in_=_bcast_rows(P["ln4b"].ap(), 0, M))

        # ---------- u_norm and its transpose ----------
        usq = consts.tile([BC, M], F32, tag="usq")
        nc.scalar.activation(out=usq[:], in_=u_sb[:], func=AF.Square)
        nrm = consts.tile([BC, 1], F32, tag="nrm")
        nc.vector.reduce_sum(out=nrm[:], in_=usq[:], axis=AX.X)
        nc.scalar.activation(out=nrm[:], in_=nrm[:], func=AF.Sqrt)
        nc.vector.tensor_scalar_max(nrm[:], nrm[:], 1e-12)
        nc.vector.reciprocal(out=nrm[:], in_=nrm[:])
        unorm = consts.tile([BC, M], F32, tag="unorm")
        nc.vector.tensor_scalar_mul(unorm[:], u_sb[:], nrm[:])
        tp = tp_ps.tile([128, BC], F32, tag="tp")
        nc.tensor.transpose(tp[:], unorm[:], ident[:BC, :BC])
        unT = consts.tile([128, BC], F32, tag="unT")
        nc.vector.tensor_copy(out=unT[:], in_=tp[:])

        # ---------- q = xc @ W_xc + u_norm @ W_u   (natural [b, f]) ----------
        qps = tp_ps.tile([BC, F], F32, tag="tp", name="qps")
        for k in range(4):
            nc.tensor.matmul(qps[:], lhsT=xT_f[:, k, :], rhs=fcxc[:, k, :],
                             start=(k == 0), stop=False)
        for k in range(8):
            nc.tensor.matmul(qps[:], lhsT=cT_f[:, k, :], rhs=fcxc[:, 4 + k, :],
                             start=False, stop=False)
        nc.tensor.matmul(qps[:], lhsT=unT[:], rhs=fcu[:],
                         start=False, stop=True)
        q_nat = consts.tile([BC, F], F32, tag="q_nat")
        nc.vector.tensor_copy(out=q_nat[:], in_=qps[:])
        # qT [f, b] for the broadcast add
        qT = consts.tile([128, 2, BC], F32, tag="qT")
        for f in range(2):
            tpq = tp_ps.tile([128, BC], F32, tag="tp")
            nc.tensor.transpose(tpq[:], q_nat[:, f * 128:(f + 1) * 128],
                                ident[:BC, :BC])
            nc.vector.tensor_copy(out=qT[:, f, :], in_=tpq[:])

        # ---------- r_km^T [f, m] = fc_kpad.T @ keysT_pad ----------
        rkT = consts.tile([128, 2, M], F32, tag="rkT")
        for f in range(2):
            rps = tp_ps.tile([128, M], F32, tag="tp", name="rps")
            nc.tensor.matmul(rps[:], lhsT=fckp[:, f * 128:(f + 1) * 128],
                             rhs=keysT[:], start=True, stop=True)
            nc.vector.tensor_copy(out=rkT[:, f, :], in_=rps[:])

        # ---------- score path main loop ----------
        for ci in range(NCHUNK):
            ht_hi = hpool.tile([128, 8, CHUNK], BF16, tag="ht_hi")
            nc.sync.dma_start(out=ht_hi[:],
                              in_=P["hmemT_hi"].ap()[:, ci * CHUNK:(ci + 1) * CHUNK]
                              .rearrange("(j p) n -> p j n", p=128))
            ht_lo = hpool.tile([128, 8, CHUNK], BF16, tag="ht_lo")
            nc.sync.dma_start(out=ht_lo[:],
                              in_=P["hmemT_lo"].ap()[:, ci * CHUNK:(ci + 1) * CHUNK]
                              .rearrange("(j p) n -> p j n", p=128))
            sps = sc_ps.tile([1, CHUNK], F32, tag="sps")
            for f in range(2):
                fs = slice(f * 128, (f + 1) * 128)
                ps = pre_ps.tile([128, CHUNK], F32, tag="pre")
                for kh in range(8):
                    nc.tensor.matmul(ps[:], lhsT=fchm_hi[:, kh, fs],
                                     rhs=ht_hi[:, kh, :],
                                     start=(kh == 0), stop=False)
                for kh in range(8):
                    nc.tensor.matmul(ps[:], lhsT=fchm_lo[:, kh, fs],
                                     rhs=ht_hi[:, kh, :], start=False, stop=False)
                for kh in range(8):
                    nc.tensor.matmul(ps[:], lhsT=fchm_hi[:, kh, fs],
                                     rhs=ht_lo[:, kh, :], start=False,
                                     stop=(kh == 7))
                hf = hfpool.tile([128, CHUNK], F32, tag="hf")
                nb = CHUNK // M  # batch rows per chunk (2)
                qb = qT[:, f, ci * nb:(ci + 1) * nb, None].to_broadcast([128, nb, M])
                nc.vector.tensor_tensor(
                    out=hf[:].rearrange("p (b m) -> p b m", b=nb),
                    in0=ps[:].rearrange("p (b m) -> p b m", b=nb),
                    in1=qb, op=ALU.add)
                rb = rkT[:, f, None, :].to_broadcast([128, nb, M])
                nc.vector.tensor_tensor(
                    out=hf[:].rearrange("p (b m) -> p b m", b=nb),
                    in0=hf[:].rearrange("p (b m) -> p b m", b=nb),
                    in1=rb, op=ALU.add)
                nc.scalar.activation(out=hf[:], in_=hf[:], func=AF.Tanh,
                                     bias=fcb[:, f:f + 1], scale=1.0)
                nc.tensor.matmul(sps[:], lhsT=veca[:, f:f + 1], rhs=hf[:],
                                 start=(f == 0), stop=(f == 1))
            scs = hfpool.tile([1, CHUNK], F32, tag="scs")
            nc.vector.tensor_copy(out=scs[:], in_=sps[:])
            nc.sync.dma_start(
                out=score_d.ap()[ci * CHUNK:(ci + 1) * CHUNK]
                .rearrange("(a n) -> a n", a=1),
                in_=scs[:])

        score_bm = consts.tile([BC, M], F32, tag="score_bm")
        nc.sync.dma_start(out=score_bm[:], in_=score_d.ap().rearrange("(b m) -> b m", b=BC))

        # score -= prev*100 ; ln4 ; + gumbel ; argmax
        p100 = consts.tile([BC, M], F32, tag="p100")
        nc.vector.tensor_scalar_mul(p100[:], prev_sb[:], 100.0)
        nc.vector.tensor_sub(out=score_bm[:], in0=score_bm[:], in1=p100[:])
        _ln_rows(nc, bnpool, score_bm[:], M, ln4g_t[:], ln4b_t[:], score_bm[:], eps_t)
        gt = consts.tile([BC, M], F32, tag="gt")
        nc.scalar.activation(out=gt[:], in_=gum_sb[:], func=AF.Ln, bias=e20_t[:])
        nc.vector.tensor_scalar(out=gt[:], in0=gt[:], scalar1=-1.0, scalar2=1e-20,
                                op0=ALU.mult, op1=ALU.add)
        nc.scalar.activation(out=gt[:], in_=gt[:], func=AF.Ln)
        nc.vector.tensor_sub(out=score_bm[:], in0=score_bm[:], in1=gt[:])
        mx8 = consts.tile([BC, 8], F32, tag="mx8")
        nc.vector.max(out=mx8[:], in_=score_bm[:])
        mi8 = consts.tile([BC, 8], U32, tag="mi8")
        nc.vector.max_index(out=mi8[:], in_max=mx8[:], in_values=score_bm[:])
        flat = consts.tile([BC, 1], U32, tag="flat")
        nc.vector.tensor_tensor(out=flat[:], in0=rowb[:], in1=mi8[:, 0:1], op=ALU.add)

        # gather h_entry rows (fp32 exact)
        h_ent = consts.tile([BC, H], F32, tag="h_ent")
        nc.gpsimd.indirect_dma_start(
            out=h_ent[:], out_offset=None, in_=P["hmem_flat"].ap(),
            in_offset=bass.IndirectOffsetOnAxis(ap=flat[:, :1], axis=0))

        # h_entry^T tiles (fp32 for gating, bf16 for matmul 1)
        hT_f = consts.tile([128, 8, BC], F32, tag="hT_f")
        for kh in range(8):
            tph = tp_ps.tile([128, BC], F32, tag="tp")
            nc.tensor.transpose(tph[:], h_ent[:, kh * 128:(kh + 1) * 128],
                                ident[:BC, :BC])
            nc.vector.tensor_copy(out=hT_f[:, kh, :], in_=tph[:])
            nc.vector.tensor_copy(out=ck_bf[:, 12 + kh, :], in_=tph[:])

        # ---------- matmul 1: z1 = concat0 @ W_full1 (+bias1), ln3, sigmoid ----------
        g1 = consts.tile([BC, 2 * H], F32, tag="g1")
        for pz in range(2):
            ps1 = cell_ps.tile([BC, 1024], F32, tag="cellps", name="ps1")
            for kq in range(5):
                w1t = wpool.tile([128, 4, 1024], BF16, tag="wt")
                nc.sync.dma_start(
                    out=w1t[:],
                    in_=P["w1bf"].ap()[kq * 512:(kq + 1) * 512,
                                       pz * 1024:(pz + 1) * 1024]
                    .rearrange("(j p) n -> p j n", p=128))
                for j in range(4):
                    k = kq * 4 + j
                    for nn in range(2):
                        nc.tensor.matmul(ps1[:, nn * 512:(nn + 1) * 512],
                                         lhsT=ck_bf[:, k, :],
                                         rhs=w1t[:, j, nn * 512:(nn + 1) * 512],
                                         start=(k == 0), stop=(k == KCAT - 1))
            z1p = zpool.tile([BC, 1024], F32, tag="z1p")
            b1c = bcpool.tile([BC, 1024], F32, tag="bc")
            nc.gpsimd.dma_start(out=b1c[:], in_=_bcast_rows(P["bias1v"].ap(),
                                                            pz * 1024, (pz + 1) * 1024))
            nc.vector.tensor_add(out=z1p[:], in0=ps1[:], in1=b1c[:])
            g3 = bcpool.tile([BC, 1024], F32, tag="bc")
            nc.gpsimd.dma_start(out=g3[:], in_=_bcast_rows(P["ln3g"].ap(),
                                                           pz * 1024, (pz + 1) * 1024))
            b3 = bcpool.tile([BC, 1024], F32, tag="bc")
            nc.gpsimd.dma_start(out=b3[:], in_=_bcast_rows(P["ln3b"].ap(),
                                                           pz * 1024, (pz + 1) * 1024))
            _ln_rows(nc, bnpool, z1p[:], 1024, g3[:], b3[:], z1p[:], eps_t)
            nc.scalar.activation(out=g1[:, pz * 1024:(pz + 1) * 1024], in_=z1p[:],
                                 func=AF.Sigmoid)

        # gate: ckg[4+t] = (cT | h_entry^T)[t] * g1^T[t]   (bf16 cast on write)
        for t in range(16):
            tpg = tp_ps.tile([128, BC], F32, tag="tp")
            nc.tensor.transpose(tpg[:], g1[:, t * 128:(t + 1) * 128], ident[:BC, :BC])
            src = cT_f[:, t, :] if t < 8 else hT_f[:, t - 8, :]
            nc.vector.tensor_mul(out=ckg_bf[:, 4 + t, :], in0=src, in1=tpg[:])

        # ---------- matmul 2: z = gated @ W_full (+bias), ln1 per chunk ----------
        zln = [consts.tile([BC, 1024], F32, tag=f"zln{i}", name=f"zln{i}")
               for i in range(5)]
        for pz in range(5):
            ps2 = cell_ps.tile([BC, 1024], F32, tag="cellps", name="ps2")
            for kq in range(5):
                wft = wpool.tile([128, 4, 1024], BF16, tag="wt")
                nc.sync.dma_start(
                    out=wft[:],
                    in_=P["wfbf"].ap()[kq * 512:(kq + 1) * 512,
                                       pz * 1024:(pz + 1) * 1024]
                    .rearrange("(j p) n -> p j n", p=128))
                for j in range(4):
                    k = kq * 4 + j
                    for nn in range(2):
                        nc.tensor.matmul(ps2[:, nn * 512:(nn + 1) * 512],
                                         lhsT=ckg_bf[:, k, :],
                                         rhs=wft[:, j, nn * 512:(nn + 1) * 512],
                                         start=(k == 0), stop=(k == KCAT - 1))
            bvc = bcpool.tile([BC, 1024], F32, tag="bc")
            nc.gpsimd.dma_start(out=bvc[:], in_=_bcast_rows(P["biasv"].ap(),
                                                            pz * 1024, (pz + 1) * 1024))
            nc.vector.tensor_add(out=zln[pz][:], in0=ps2[:], in1=bvc[:])
            g1c = bcpool.tile([BC, 1024], F32, tag="bc")
            nc.gpsimd.dma_start(out=g1c[:], in_=_bcast_rows(P["ln1g"].ap(),
                                                            pz * 1024, (pz + 1) * 1024))
            b1cc = bcpool.tile([BC, 1024], F32, tag="bc")
            nc.gpsimd.dma_start(out=b1cc[:], in_=_bcast_rows(P["ln1b"].ap(),
                                                             pz * 1024, (pz + 1) * 1024))
            _ln_rows(nc, bnpool, zln[pz][:], 1024, g1c[:], b1cc[:], zln[pz][:], eps_t)

        # ---------- cell math ----------
        zi, zj, zf, zo, zom = zln
        nc.scalar.activation(out=zf[:], in_=zf[:], func=AF.Sigmoid, bias=F_BIAS)
        nc.scalar.activation(out=zi[:], in_=zi[:], func=AF.Sigmoid)
        nc.scalar.activation(out=zj[:], in_=zj[:], func=AF.Tanh)
        nc.vector.tensor_mul(out=zf[:], in0=c_nat[:], in1=zf[:])
        nc.vector.tensor_mul(out=zi[:], in0=zi[:], in1=zj[:])
        nc.vector.tensor_add(out=zf[:], in0=zf[:], in1=zi[:])
        g2c = bcpool.tile([BC, H], F32, tag="bc")
        nc.gpsimd.dma_start(out=g2c[:], in_=_bcast_rows(P["ln2g"].ap(), 0, H))
        b2c = bcpool.tile([BC, H], F32, tag="bc")
        nc.gpsimd.dma_start(out=b2c[:], in_=_bcast_rows(P["ln2b"].ap(), 0, H))
        _ln_rows(nc, bnpool, zf[:], H, g2c[:], b2c[:], zf[:], eps_t)
        nc.scalar.activation(out=zj[:], in_=zf[:], func=AF.Tanh)
        nc.scalar.activation(out=zo[:], in_=zo[:], func=AF.Sigmoid)
        nc.vector.tensor_mul(out=zj[:], in0=zj[:], in1=zo[:])
        rh = consts.tile([BC, H], F32, tag="rh")
        nc.scalar.activation(out=rh[:], in_=h_ent[:], func=AF.Tanh)
        nc.scalar.activation(out=zom[:], in_=zom[:], func=AF.Sigmoid)
        nc.vector.tensor_mul(out=rh[:], in0=rh[:], in1=zom[:])

        nc.sync.dma_start(out=out_d.ap()[:, 0:H], in_=zj[:])
        nc.sync.dma_start(out=out_d.ap()[:, H:2 * H], in_=rh[:])

    nc.compile()
    return nc


_NC = None


def _get_nc():
    global _NC
    if _NC is None:
        _NC = build_nc()
    return _NC


def make_in_maps(inputs):
    inp = {k: np.asarray(v) for k, v in inputs.items()}
    x = inp["x"].astype(np.float32)
    c = inp["c"].astype(np.float32)
    hmem = inp["hmem"].astype(np.float32)
    bf = ml_dtypes.bfloat16

    keysT_pad = np.zeros((128, M), np.float32)
    keysT_pad[:KD] = inp["keys"].astype(np.float32).T
    row_base = (np.arange(BC, dtype=np.uint32) * M).reshape(BC, 1)
    w1bf = inp["W_full1"].astype(bf)
    wfbf = inp["W_full"].astype(bf)

    shared = dict(
        keysT_pad=keysT_pad, row_base=row_base,
        fc_w=inp["fc_w"].astype(np.float32),
        fchm_hi=None, fchm_lo=None,
        fc_b=inp["fc_b"].astype(np.float32),
        vec_a=inp["vec_a"].astype(np.float32).reshape(F),
        w1bf=w1bf, wfbf=wfbf,
        bias1v=inp["bias1"].astype(np.float32),
        biasv=inp["bias"].astype(np.float32),
        ln1g=inp["ln1_g"].astype(np.float32), ln1b=inp["ln1_b"].astype(np.float32),
        ln2g=inp["ln2_g"].astype(np.float32), ln2b=inp["ln2_b"].astype(np.float32),
        ln3g=inp["ln3_g"].astype(np.float32), ln3b=inp["ln3_b"].astype(np.float32),
        ln4g=inp["ln4_g"].astype(np.float32), ln4b=inp["ln4_b"].astype(np.float32),
    )

    whm = inp["fc_w"].astype(np.float32)[1600:2624, :]
    whm_hi = whm.astype(bf)
    shared["fchm_hi"] = whm_hi
    shared["fchm_lo"] = (whm - whm_hi.astype(np.float32)).astype(bf)

    in_maps = []
    for cid in range(N_CORES):
        b0 = cid * BC
        xs = x[b0:b0 + BC]
        cs = c[b0:b0 + BC]
        hs = hmem[b0:b0 + BC]                              # [BC, M, H]
        m = dict(shared)
        hT = np.ascontiguousarray(hs.transpose(2, 0, 1).reshape(H, BM))
        hT_hi = hT.astype(bf)
        m["hmemT_hi"] = hT_hi
        m["hmemT_lo"] = (hT - hT_hi.astype(np.float32)).astype(bf)
        m["hmem_flat"] = np.ascontiguousarray(hs.reshape(BM, H))
        m["xT"] = np.ascontiguousarray(xs.T)
        m["cT"] = np.ascontiguousarray(cs.T)
        m["xT_bf"] = np.ascontiguousarray(xs.T).astype(bf)
        m["cT_bf"] = np.ascontiguousarray(cs.T).astype(bf)
        m["c_nat"] = cs
        m["u_t"] = inp["u_t"][b0:b0 + BC].astype(np.float32)
        m["prev"] = inp["prev_read_location"][b0:b0 + BC].astype(np.float32)
        m["gumbel_u"] = inp["gumbel_u"][b0:b0 + BC].astype(np.float32)
        in_maps.append(m)
    return in_maps


_LAST_EXEC_NS = None


def kernel(**inputs):
    global _LAST_EXEC_NS
    import os
    nc = _get_nc()
    in_maps = make_in_maps(inputs)
    trace = bool(int(os.environ.get("KERNEL_TRACE", "0")))
    res = run_bass_kernel_spmd(nc, in_maps, list(range(N_CORES)), trace=trace)
    _LAST_EXEC_NS = res.exec_time_ns
    out = np.concatenate([res.results[i]["out"] for i in range(N_CORES)], axis=0)
    return out.astype(np.float32)
